# revision 32
# baseline (speedup 1.0000x reference)
"""Binary conv + BN(train) + ReLU fused Trainium2 SPMD kernel.

Reference computation (NCHW, x:(32,256,56,56) f32):
    mean/var over (N,H,W) per channel; xn = (x-mean)*rsqrt(var+eps)*gamma+beta
    xb = sign(xn); wb = sign(W); y = relu(conv3x3(xb, wb, pad=1) + bias)

Strategy: data-parallel over batch across 8 NeuronCores (4 images each).

The cross-core reduction of the 2KB BN partial stats uses the CC
AllReduce. The collective firmware has a ~55-60us cold-start from kernel
launch before the first mesh can begin (measured; independent of trigger
time), which pins the conv start at ~70us+; a warm-up dummy collective
does not help (the real mesh queues behind it) and remote_dma_broadcast
hangs in this environment, so the cold start is accepted as a floor and
everything before/after it is compressed instead.

Per-core timeline: [x load (fp16, fine pieces on one issue ring so both
stats streams land early) with DVE bn_stats + ScalarE Identity/Square
accumulates trailing; stats finish ~40us, trigger ~43us, safely before
the collective window] -> [AllReduce mesh ~62-80us] -> [result on the
idle sync queue -> warmed threshold chain, mesh-end to first matmul
~4.5us] -> [sign split: ScalarE does chunk 0 via Sign activation (+-1
fp8), DVE does chunk 1 via (x>thr)-0.5 (+-0.5 fp8, weights host-scaled
x2 -> products stay +-1 exactly), fine first slices so the conv unblocks
ASAP] -> [3x3 conv as 9 accumulating DoubleRow fp8 matmuls (K=256) per
128x448 psum tile, image-major weight-reuse blocks with LDWEIGHTS
elided, gap-free at ~197ns/matmul] -> [bias+relu fused into the PSUM
drain, fp16 stores (host casts back to f32), 1-job tail block]. Sign
values are exact in fp8 and PSUM accumulates in fp32, so the binarized
conv is exact up to fp16 x rounding (rel err ~7e-3 << 2e-2).
"""

import sys

for _p in ("/opt/trn_rl_repo", "/root/.axon_site/_ro/trn_rl_repo"):
    if _p not in sys.path:
        sys.path.append(_p)

import numpy as np

import concourse.bass as bass
from concourse.bass import ds
import concourse.mybir as mybir
import concourse.tile as tile
from concourse import bacc, bass_utils

F32 = mybir.dt.float32
F16 = mybir.dt.float16
FP8 = mybir.dt.float8e4
AF = mybir.ActivationFunctionType
ALU = mybir.AluOpType

N_CORES = 8
NB = 4          # images per core
C = 256
P = 128         # partitions / chunk size
NCH = 2         # channel chunks (ci and co)
H = W = 56
HW = H * W      # 3136
PH = PW = 58    # padded plane
PSZ = PH * PW   # 3364
RG = 8          # output rows per psum tile
NG = H // RG    # 7 row groups
NT = RG * W     # 448 columns per matmul
BN_EPS = 1e-5
N_WARM_MM = 16  # PE warm-up matmuls during the stats phase

_CACHE = {}


def _build_nc():
    nc = bacc.Bacc("TRN2", target_bir_lowering=False, debug=False,
                   num_devices=N_CORES)
    # x pre-converted to fp16 on the host: halves the critical-path HBM read
    # and doubles stats/sign throughput; sign() absorbs the 2^-11 rounding.
    xs = nc.dram_tensor("xs", [NB, C, H, W], F16, kind="ExternalInput")
    wt = nc.dram_tensor("wt", [P, NCH, 9 * NCH * P], FP8, kind="ExternalInput")
    par = nc.dram_tensor("par", [NCH, P, 3], F32, kind="ExternalInput")
    ys = nc.dram_tensor("ys", [NB, C, H, W], F16, kind="ExternalOutput")
    # Shared-output collective is the HBM-HBM RDH fast path for 8-core groups
    cc_sh = nc.dram_tensor("cc_sh", [N_CORES, P, 2 * NCH], F32, kind="Internal",
                           addr_space="Shared")

    with tile.TileContext(nc) as tc:
        with (
            tc.tile_pool(name="main", bufs=1) as main,
            tc.tile_pool(name="outp", bufs=4) as outp,
            tc.tile_pool(name="psum", bufs=8, space="PSUM") as psum,
            tc.tile_pool(name="dram", bufs=1, space="DRAM") as dram,
        ):
            xt = [main.tile([P, NB * HW], F16, name=f"xt{c}") for c in range(NCH)]
            # sign planes: [p, ci_chunk, image, padded 58x58] (chunk dim = fp8
            # DoubleRow pair dim)
            xball = main.tile([P, NCH, NB * PSZ], FP8, name="xball")
            xbv = xball.rearrange("p j (n h w) -> p j n h w", n=NB, h=PH)
            wb = main.tile([P, NCH, 9 * NCH * P], FP8, name="wb")
            parc = main.tile([P, 3 * NCH], F32, name="parc")  # [r,beta,bias] x chunk

            # ---- x load: half-image pieces (keeps ~16 transfers in flight
            # for aggregate HBM bandwidth); chunk 0 on the sync queue,
            # chunk 1 on the scalar queue. Last image in single-group
            # pieces so stats can trail the tail closely.
            def _xdma(eng, n, c, g0, g1):
                eng.dma_start(
                    xt[c][:, n * HW + g0 * NT:n * HW + g1 * NT],
                    xs[n, c * P:(c + 1) * P]
                    .rearrange("p h w -> p (h w)")[:, g0 * NT:g1 * NT],
                )

            # weights/params first: the x load has ~30us of slack before the
            # collective window, and early weights let the conv's LDWEIGHTS
            # path warm up
            nc.sync.dma_start(wb[:], wt[:])
            nc.sync.dma_start(
                parc.rearrange("p (c s) -> p c s", s=3),
                par.rearrange("c p s -> p c s"),
            )
            # all x on the sync ring, c0/c1 interleaved: one ring keeps
            # strict arrival order (16 engines share it at ~350GB/s), so
            # both stats engines see their streams early; the scalar queue
            # issues nothing and ScalarE stats start as soon as data lands
            for n in range(NB - 1):
                for g0, g1 in ((0, 1), (1, 3), (3, 5), (5, NG)):
                    for c in range(NCH):
                        _xdma(nc.sync, n, c, g0, g1)
            for g in range(NG):
                for c in range(NCH):
                    _xdma(nc.sync, NB - 1, c, g, g + 1)

            # eps constant for the fused Sqrt(var + eps) activation bias
            epsT = main.tile([P, 1], F32, name="epsT")
            nc.gpsimd.memset(epsT[:], BN_EPS)

            # zero only the pad borders of the sign planes (GpSimd; interior
            # is fully overwritten by the sign ops)
            for c in range(NCH):
                for n in range(NB):
                    nc.gpsimd.memset(xbv[:, c, n, 0, :], 0.0)
                    nc.gpsimd.memset(xbv[:, c, n, PH - 1, :], 0.0)
                    nc.gpsimd.memset(xbv[:, c, n, 1:PH - 1, 0], 0.0)
                    nc.gpsimd.memset(xbv[:, c, n, 1:PH - 1, PW - 1], 0.0)

            # ---- one-pass partial stats trailing the DMA.
            # DVE: bn_stats (<=512 cols/instr) over chunk 0 + the late-
            # arriving chunk-1 tail; ScalarE: Identity/Square accumulates
            # over chunk-1 images 0-2 (first 4 groups of image 2).
            def _xp(c, n, g0, g1):
                return xt[c][:, n * HW + g0 * NT:n * HW + g1 * NT]

            st6a = main.tile([P, NB * NG * 6], F32, name="st6a")
            st6b = main.tile([P, NG * 6], F32, name="st6b")
            dve_pieces = [(0, n, g) for n in range(3) for g in range(NG)]
            for g in range(NG):
                dve_pieces.append((0, NB - 1, g))
                dve_pieces.append((1, NB - 1, g))
            sl_a = sl_b = 0
            for c, n, g in dve_pieces:
                if c == 0:
                    dst = st6a[:, sl_a * 6:(sl_a + 1) * 6]
                    sl_a += 1
                else:
                    dst = st6b[:, sl_b * 6:(sl_b + 1) * 6]
                    sl_b += 1
                nc.vector.bn_stats(dst, _xp(c, n, g, g + 1))
            NTAIL = float(NG * NT)  # cols covered by st6b

            scrS = main.tile([P, HW], F16, name="scrS")
            sq_s = main.tile([P, 12], F32, name="sq_s")  # [s,q] x 6 pieces
            s_pieces = [(n, g0, g1) for n in range(3)
                        for g0, g1 in ((0, 4), (4, NG))]
            for i, (n, g0, g1) in enumerate(s_pieces):
                w_ = (g1 - g0) * NT
                nc.scalar.activation(scrS[:, :w_], _xp(1, n, g0, g1),
                                     AF.Identity,
                                     accum_out=sq_s[:, 2 * i:2 * i + 1])
                nc.scalar.activation(scrS[:, :w_], _xp(1, n, g0, g1),
                                     AF.Square,
                                     accum_out=sq_s[:, 2 * i + 1:2 * i + 2])

            # ScalarE activation-table warm-up (Sqrt/Sign/Relu) after its
            # stats, so no ACT_TABLE_LOAD lands in the post-exchange
            # critical path
            wmf = main.tile([P, 1], F32, name="wmf")
            wm8 = main.tile([P, 1], FP8, name="wm8")
            nc.scalar.activation(wmf[:], epsT[:], AF.Sqrt, bias=epsT[:, 0:1])
            nc.scalar.activation(wm8[:], epsT[:], AF.Sign, bias=epsT[:, 0:1])
            nc.scalar.activation(wmf[:], epsT[:], AF.Relu, bias=epsT[:, 0:1])

            # per-core (mean, E[x^2])/8 partials; layout pre = [m0,e0,m1,e1]
            pre = main.tile([P, 2 * NCH], F32)
            mv = main.tile([P, 2], F32)
            mv3 = main.tile([P, 2], F32)
            t_a = main.tile([P, 1], F32)
            u_a = main.tile([P, 3, 2], F32)
            u_s = main.tile([P, 2], F32)  # [Sx, Sx^2] accumulators (chunk1)
            u_w = main.tile([P, 1], F32)
            SCL = 1.0 / (NB * HW * N_CORES)
            nc.vector.bn_aggr(mv[:], st6a[:])
            nc.vector.tensor_mul(t_a[:], mv[:, 0:1], mv[:, 0:1])
            nc.vector.tensor_scalar(
                pre[:, 1:2], mv[:, 1:2], t_a[:], 1.0 / N_CORES,
                op0=ALU.add, op1=ALU.mult,
            )
            nc.vector.tensor_scalar_mul(pre[:, 0:1], mv[:, 0:1], 1.0 / N_CORES)
            nc.vector.bn_aggr(mv3[:], st6b[:])
            sqv = sq_s.rearrange("p (n k) -> p n k", k=2)
            nc.vector.tensor_add(u_a[:], sqv[:, 0:3, :], sqv[:, 3:6, :])
            nc.vector.tensor_add(u_s[:], u_a[:, 0, :], u_a[:, 1, :])
            nc.vector.tensor_add(u_s[:], u_s[:], u_a[:, 2, :])
            # Sx_c1 += mean_b*NTAIL; Sq_c1 += (var_b+mean_b^2)*NTAIL
            nc.vector.scalar_tensor_tensor(
                u_s[:, 0:1], mv3[:, 0:1], NTAIL, u_s[:, 0:1],
                op0=ALU.mult, op1=ALU.add,
            )
            nc.vector.scalar_tensor_tensor(
                u_w[:], mv3[:, 0:1], mv3[:, 0:1], mv3[:, 1:2],
                op0=ALU.mult, op1=ALU.add,
            )
            nc.vector.scalar_tensor_tensor(
                u_s[:, 1:2], u_w[:], NTAIL, u_s[:, 1:2],
                op0=ALU.mult, op1=ALU.add,
            )
            nc.vector.tensor_scalar_mul(pre[:, 2:3], u_s[:, 0:1], SCL)
            nc.vector.tensor_scalar_mul(pre[:, 3:4], u_s[:, 1:2], SCL)
            # pre holds per-core (mean, E[x^2]) per chunk; the exchange sums
            # across cores, so scale by 1/8 AFTER the sum (done via SCL8 in
            # the threshold math below on the summed values /8).

            # ---- cross-core sum of `pre` via a CC AllGather + local
            # tree-add: an AllReduce is ReduceScatter+AllGather (2(N-1)
            # latency-bound ring steps for 2KB), a bare AllGather is (N-1)
            # steps, so the mesh is ~half the duration; summing the 8
            # gathered 2KB contributions locally costs 3 DVE adds. The
            # collective firmware still has a ~55-60us cold start from
            # kernel launch before the first mesh can begin, independent
            # of the trigger time (measured; a warm-up dummy collective
            # does not help, the real mesh just queues behind it;
            # remote_dma_broadcast hangs in this environment).
            cc_in = dram.tile([P, 2 * NCH], F32)
            nc.gpsimd.dma_start(cc_in[:], pre[:])
            nc.gpsimd.collective_compute(
                "AllGather",
                ALU.bypass,
                replica_groups=[list(range(N_CORES))],
                ins=[cc_in[:].opt()],
                outs=[cc_sh[:].opt()],
            )
            gsr = main.tile([P, N_CORES, 2 * NCH], F32)
            # result load on the sync queue (idle here; HWDGE beats the
            # gpsimd SWDGE issue latency)
            nc.sync.dma_start(gsr[:], cc_sh.rearrange("r p s -> p r s"))
            gs = main.tile([P, 2 * NCH], F32)
            tr0 = main.tile([P, 4, 2 * NCH], F32)
            tr1 = main.tile([P, 2, 2 * NCH], F32)
            nc.vector.tensor_add(tr0[:], gsr[:, 0:4, :], gsr[:, 4:8, :])
            nc.vector.tensor_add(tr1[:], tr0[:, 0:2, :], tr0[:, 2:4, :])
            nc.vector.tensor_add(gs.rearrange("p (a b) -> p a b", a=1),
                                 tr1[:, 0:1, :], tr1[:, 1:2, :])

            # thresholds: sign(gamma*(x-m)*rsqrt + beta) =
            # sign(gamma) * sign(x + (r*sigma - m)), r = beta/gamma
            # (host folds sign(gamma) into the weights, ships r in parc
            # slot 0). ab = r*sigma - m (ScalarE bias form); thr = -ab
            # (DVE compare form). gs holds the global (mean, E[x^2]).
            ab = main.tile([P, NCH], F32)
            thr = main.tile([P, NCH], F32)
            u1 = main.tile([P, NCH], F32)
            u2 = main.tile([P, NCH], F32)
            gsv = gs.rearrange("p (c s) -> p c s", s=2)
            gmean = gsv[:, :, 0]
            parv = parc.rearrange("p (c s) -> p c s", s=3)
            nc.vector.tensor_mul(u1[:], gmean, gmean)
            nc.vector.tensor_sub(u2[:], gsv[:, :, 1], u1[:])  # global var
            nc.scalar.activation(u1[:], u2[:], AF.Sqrt, bias=epsT[:, 0:1])
            nc.vector.tensor_mul(u2[:], parv[:, :, 0], u1[:])  # r*sigma
            nc.vector.tensor_sub(ab[:], u2[:], gmean)
            nc.vector.tensor_sub(thr[:], gmean, u2[:])

            # ---- normalize + sign -> padded planes, split by chunk:
            # ScalarE: chunk 0 via Sign activation (+-1)
            # DVE: chunk 1 via (x>thr)-0.5 (+-0.5; weights shipped x2).
            # Fine first slices so the first conv matmuls unblock ASAP.
            for n in range(NB):
                slices = ((0, 10), (10, 34), (34, H)) if n == 0 else ((0, 34), (34, H))
                for r0, r1 in slices:
                    nc.scalar.activation(
                        xbv[:, 0, n, 1 + r0:1 + r1, 1:1 + W],
                        xt[0][:, n * HW + r0 * W:n * HW + r1 * W]
                        .rearrange("p (h w) -> p h w", w=W),
                        AF.Sign,
                        bias=ab[:, 0:1],
                    )
                    nc.vector.tensor_scalar(
                        xbv[:, 1, n, 1 + r0:1 + r1, 1:1 + W],
                        xt[1][:, n * HW + r0 * W:n * HW + r1 * W]
                        .rearrange("p (h w) -> p h w", w=W),
                        thr[:, 1:2], 0.5,
                        op0=ALU.is_gt, op1=ALU.subtract,
                    )

            # ---- 3x3 binary conv, image-major block order (each image's
            # planes feed 14 jobs before the next image's are needed).
            # 7-job blocks leave one spare PSUM bank for block overlap.
            blocks = []
            for n in range(NB):
                for o in range(NCH):
                    jb = [(n, g) for g in range(NG)]
                    if n == 0 and o == 0:
                        # split the first block finely: [g0] runs off sign
                        # slice (0,10) alone, [g1-g3] off (10,34), the rest
                        # off (34,56)
                        blocks.append((o, jb[:1]))
                        blocks.append((o, jb[1:4]))
                        blocks.append((o, jb[4:]))
                    elif n == NB - 1 and o == NCH - 1:
                        # split the final block so its first drains+stores
                        # overlap the last jobs' matmuls, with a 1-job tail
                        blocks.append((o, jb[:4]))
                        blocks.append((o, jb[4:6]))
                        blocks.append((o, jb[6:]))
                    else:
                        blocks.append((o, jb))
            last_blk_idx = len(blocks) - 1
            for bi, (o, blk) in enumerate(blocks):
                pts = [psum.tile([P, NT], F32, name="ps", tag="ps") for _ in blk]
                for t in range(9):
                    ky, kx = divmod(t, 3)
                    w_ap = wb[:, :, (t * NCH + o) * P:(t * NCH + o + 1) * P]
                    for k, (n, g) in enumerate(blk):
                        rhs = xbv[:, :, n, g * RG + ky: g * RG + ky + RG, kx:kx + W]
                        mm = nc.tensor.matmul(
                            pts[k][:], w_ap, rhs,
                            start=(t == 0), stop=(t == 8),
                            perf_mode=mybir.MatmulPerfMode.DoubleRow,
                        )
                        if k > 0:
                            mm.ins.ldweights = False
                for k, (n, g) in enumerate(blk):
                    ob = outp.tile([P, NT], F16, name="ob", tag="ob")
                    # drains on ScalarE (own PSUM port) except the final
                    # block, where splitting across both engines halves
                    # the serialized drain tail
                    if bi == last_blk_idx:
                        nc.scalar.activation(ob[:, :NT // 2],
                                             pts[k][:, :NT // 2], AF.Relu,
                                             bias=parc[:, 3 * o + 2:3 * o + 3])
                        nc.vector.tensor_scalar(
                            ob[:, NT // 2:], pts[k][:, NT // 2:],
                            parc[:, 3 * o + 2:3 * o + 3], 0.0,
                            op0=ALU.add, op1=ALU.max,
                        )
                    else:
                        nc.scalar.activation(ob[:, :], pts[k][:, :], AF.Relu,
                                             bias=parc[:, 3 * o + 2:3 * o + 3])
                    # flat contiguous dest so the DMA coalesces full-line
                    # writes; single queue (dual-queue raced the drains)
                    nc.sync.dma_start(
                        ys[n, o * P:(o + 1) * P]
                        .rearrange("p h w -> p (h w)")[:, g * NT:(g + 1) * NT],
                        ob[:],
                    )
    nc.compile()
    return nc


def _get_nc():
    if "nc" not in _CACHE:
        _CACHE["nc"] = _build_nc()
    return _CACHE["nc"]


def _prep_inputs(x, gamma, beta, weight, bias):
    # fold sign(gamma) (per input channel) into the binarized weights so the
    # device computes just sign(x - t); r = beta/gamma feeds the threshold
    gamma = np.asarray(gamma, dtype=np.float32)
    beta = np.asarray(beta, dtype=np.float32)
    sg = np.sign(gamma)
    r = np.divide(beta, gamma, out=np.zeros_like(beta), where=gamma != 0)
    wsign = np.sign(weight.astype(np.float32)) * sg[None, :, None, None]
    # x2 on weights whose input channels are DVE-signed (+-0.5 encoding):
    # all of chunk 1
    fac = np.ones((NCH, P), dtype=np.float32)
    fac[1, :] = 2.0
    wT = (
        wsign.reshape(NCH, P, NCH, P, 3, 3)      # o, m, c, p, ky, kx
        * fac[None, None, :, :, None, None]
    )
    wT = (
        wT.transpose(3, 2, 4, 5, 0, 1)           # p, c, ky, kx, o, m
        .reshape(P, NCH, 9 * NCH * P)
        .astype(mybir.dt.np(FP8))
    )
    par = np.stack(
        [r, beta, bias.astype(np.float32)],
        axis=-1,
    ).reshape(NCH, P, 3)
    x = np.ascontiguousarray(np.asarray(x, dtype=np.float32).astype(np.float16))
    return [
        {"xs": x[j * NB:(j + 1) * NB], "wt": wT, "par": par}
        for j in range(N_CORES)
    ]


def _run(x, gamma, beta, weight, bias, trace=False, trace_cores=None):
    nc = _get_nc()
    in_maps = _prep_inputs(x, gamma, beta, weight, bias)
    res = bass_utils.run_bass_kernel_spmd(
        nc, in_maps, core_ids=list(range(N_CORES)), trace=trace,
        trace_cores=trace_cores,
    )
    out = np.concatenate([res.results[j]["ys"] for j in range(N_CORES)], axis=0)
    return out.astype(np.float32), res


def kernel(x, gamma, beta, weight, bias):
    out, _ = _run(x, gamma, beta, weight, bias, trace=False)
    return out


# revision 34
# speedup vs baseline: 1.2446x; 1.2446x over previous
"""Binary conv + BN(train) + ReLU fused Trainium2 SPMD kernel.

Reference computation (NCHW, x:(32,256,56,56) f32):
    mean/var over (N,H,W) per channel; xn = (x-mean)*rsqrt(var+eps)*gamma+beta
    xb = sign(xn); wb = sign(W); y = relu(conv3x3(xb, wb, pad=1) + bias)

Strategy: data-parallel over batch across 8 NeuronCores (4 images each).

The cross-core reduction of the 2KB BN partial stats uses the CC
AllReduce. The collective firmware has a ~55-60us cold-start from kernel
launch before the first mesh can begin (measured; independent of trigger
time), which pins the conv start at ~70us+; a warm-up dummy collective
does not help (the real mesh queues behind it) and remote_dma_broadcast
hangs in this environment, so the cold start is accepted as a floor and
everything before/after it is compressed instead.

Per-core timeline: [x load (fp16, fine pieces on the sync ring so both
stats streams land early) with DVE bn_stats + ScalarE Identity/Square
accumulates trailing; trigger ~43us, safely before the collective
window] -> [AllReduce mesh] -> [result on the idle sync queue -> warmed
threshold chain, mesh-end to first matmul ~4.5us] -> [sign split:
ScalarE does chunk 0 via Sign activation (+-1 fp8), DVE does chunk 1 via
(x>thr)-0.5 (+-0.5 fp8, weights host-scaled x2 -> products stay +-1
exactly), fine first slices so the conv unblocks ASAP] -> [3x3 conv as 9
accumulating DoubleRow fp8 matmuls (K=256) per 128x448 psum tile,
image-major weight-reuse blocks with LDWEIGHTS elided, gap-free] ->
[bias+relu fused into the PSUM drain, fp16 stores (host casts back to
f32), 1-job tail block with the drain split across both engines]. Sign values are exact in fp8 and
PSUM accumulates in fp32, so the binarized conv is exact up to fp16 x
rounding (rel err ~7e-3 << 2e-2).
"""

import os
import sys

# The collective firmware's trigger-wake latency degrades monotonically
# across successive executions on this device (measured 14 -> 77us over six
# runs) and a core reset at runtime init restores it (and the full clock);
# request one for every fresh process. setdefault so an explicit env wins.
os.environ.setdefault("NEURON_RT_RESET_CORES", "1")

for _p in ("/opt/trn_rl_repo", "/root/.axon_site/_ro/trn_rl_repo"):
    if _p not in sys.path:
        sys.path.append(_p)

import numpy as np

import concourse.bass as bass
from concourse.bass import ds
import concourse.mybir as mybir
import concourse.tile as tile
from concourse import bacc, bass_utils

F32 = mybir.dt.float32
F16 = mybir.dt.float16
FP8 = mybir.dt.float8e4
AF = mybir.ActivationFunctionType
ALU = mybir.AluOpType

N_CORES = 8
NB = 4          # images per core
C = 256
P = 128         # partitions / chunk size
NCH = 2         # channel chunks (ci and co)
H = W = 56
HW = H * W      # 3136
PH = PW = 58    # padded plane
PSZ = PH * PW   # 3364
RG = 8          # output rows per psum tile
NG = H // RG    # 7 row groups
NT = RG * W     # 448 columns per matmul
BN_EPS = 1e-5

_CACHE = {}


def _build_nc():
    nc = bacc.Bacc("TRN2", target_bir_lowering=False, debug=False,
                   num_devices=N_CORES)
    # x pre-converted to fp16 on the host: halves the critical-path HBM read
    # and doubles stats/sign throughput; sign() absorbs the 2^-11 rounding.
    xs = nc.dram_tensor("xs", [NB, C, H, W], F16, kind="ExternalInput")
    wt = nc.dram_tensor("wt", [P, NCH, 9 * NCH * P], FP8, kind="ExternalInput")
    par = nc.dram_tensor("par", [NCH, P, 3], F32, kind="ExternalInput")
    ys = nc.dram_tensor("ys", [NB, C, H, W], F16, kind="ExternalOutput")
    # Shared-output AllReduce is the HBM-HBM RDH fast path for 8-core groups
    cc_sh = nc.dram_tensor("cc_sh", [P, 2 * NCH], F32, kind="Internal",
                           addr_space="Shared")

    with tile.TileContext(nc) as tc:
        with (
            tc.tile_pool(name="main", bufs=1) as main,
            tc.tile_pool(name="outp", bufs=4) as outp,
            tc.tile_pool(name="psum", bufs=8, space="PSUM") as psum,
            tc.tile_pool(name="dram", bufs=1, space="DRAM") as dram,
        ):
            xt = [main.tile([P, NB * HW], F16, name=f"xt{c}") for c in range(NCH)]
            # sign planes: [p, ci_chunk, image, padded 58x58] (chunk dim = fp8
            # DoubleRow pair dim)
            xball = main.tile([P, NCH, NB * PSZ], FP8, name="xball")
            xbv = xball.rearrange("p j (n h w) -> p j n h w", n=NB, h=PH)
            wb = main.tile([P, NCH, 9 * NCH * P], FP8, name="wb")
            parc = main.tile([P, 3 * NCH], F32, name="parc")  # [r,beta,bias] x chunk

            # ---- x load: half-image pieces (keeps ~16 transfers in flight
            # for aggregate HBM bandwidth); chunk 0 on the sync queue,
            # chunk 1 on the scalar queue. Last image in single-group
            # pieces so stats can trail the tail closely.
            def _xdma(eng, n, c, g0, g1):
                eng.dma_start(
                    xt[c][:, n * HW + g0 * NT:n * HW + g1 * NT],
                    xs[n, c * P:(c + 1) * P]
                    .rearrange("p h w -> p (h w)")[:, g0 * NT:g1 * NT],
                )

            # weights/params first: the x load has ~30us of slack before the
            # collective window, and early weights let the conv's LDWEIGHTS
            # path warm up
            nc.sync.dma_start(wb[:], wt[:])
            nc.sync.dma_start(
                parc.rearrange("p (c s) -> p c s", s=3),
                par.rearrange("c p s -> p c s"),
            )
            # all x on the sync ring, c0/c1 interleaved: one ring keeps
            # strict arrival order (16 engines share it at ~350GB/s), so
            # both stats engines see their streams early; the scalar queue
            # issues nothing and ScalarE stats start as soon as data lands
            for n in range(NB - 1):
                for g0, g1 in ((0, 1), (1, 3), (3, 5), (5, NG)):
                    for c in range(NCH):
                        _xdma(nc.sync, n, c, g0, g1)
            for g in range(NG):
                for c in range(NCH):
                    _xdma(nc.sync, NB - 1, c, g, g + 1)

            # eps constant for the fused Sqrt(var + eps) activation bias
            epsT = main.tile([P, 1], F32, name="epsT")
            nc.gpsimd.memset(epsT[:], BN_EPS)

            # zero only the pad borders of the sign planes (GpSimd; interior
            # is fully overwritten by the sign ops)
            for c in range(NCH):
                for n in range(NB):
                    nc.gpsimd.memset(xbv[:, c, n, 0, :], 0.0)
                    nc.gpsimd.memset(xbv[:, c, n, PH - 1, :], 0.0)
                    nc.gpsimd.memset(xbv[:, c, n, 1:PH - 1, 0], 0.0)
                    nc.gpsimd.memset(xbv[:, c, n, 1:PH - 1, PW - 1], 0.0)

            # ---- one-pass partial stats trailing the DMA.
            # DVE: bn_stats (<=512 cols/instr) over chunk 0 + the late-
            # arriving chunk-1 tail; ScalarE: Identity/Square accumulates
            # over chunk-1 images 0-2 (first 4 groups of image 2).
            def _xp(c, n, g0, g1):
                return xt[c][:, n * HW + g0 * NT:n * HW + g1 * NT]

            st6a = main.tile([P, NB * NG * 6], F32, name="st6a")
            st6b = main.tile([P, NG * 6], F32, name="st6b")
            dve_pieces = [(0, n, g) for n in range(3) for g in range(NG)]
            for g in range(NG):
                dve_pieces.append((0, NB - 1, g))
                dve_pieces.append((1, NB - 1, g))
            sl_a = sl_b = 0
            for c, n, g in dve_pieces:
                if c == 0:
                    dst = st6a[:, sl_a * 6:(sl_a + 1) * 6]
                    sl_a += 1
                else:
                    dst = st6b[:, sl_b * 6:(sl_b + 1) * 6]
                    sl_b += 1
                nc.vector.bn_stats(dst, _xp(c, n, g, g + 1))
            NTAIL = float(NG * NT)  # cols covered by st6b

            scrS = main.tile([P, HW], F16, name="scrS")
            sq_s = main.tile([P, 12], F32, name="sq_s")  # [s,q] x 6 pieces
            s_pieces = [(n, g0, g1) for n in range(3)
                        for g0, g1 in ((0, 4), (4, NG))]
            for i, (n, g0, g1) in enumerate(s_pieces):
                w_ = (g1 - g0) * NT
                nc.scalar.activation(scrS[:, :w_], _xp(1, n, g0, g1),
                                     AF.Identity,
                                     accum_out=sq_s[:, 2 * i:2 * i + 1])
                nc.scalar.activation(scrS[:, :w_], _xp(1, n, g0, g1),
                                     AF.Square,
                                     accum_out=sq_s[:, 2 * i + 1:2 * i + 2])

            # ScalarE activation-table warm-up (Sqrt/Sign/Relu) after its
            # stats, so no ACT_TABLE_LOAD lands in the post-exchange
            # critical path
            wmf = main.tile([P, 1], F32, name="wmf")
            wm8 = main.tile([P, 1], FP8, name="wm8")
            nc.scalar.activation(wmf[:], epsT[:], AF.Sqrt, bias=epsT[:, 0:1])
            nc.scalar.activation(wm8[:], epsT[:], AF.Sign, bias=epsT[:, 0:1])
            nc.scalar.activation(wmf[:], epsT[:], AF.Relu, bias=epsT[:, 0:1])

            # per-core (mean, E[x^2])/8 partials; layout pre = [m0,e0,m1,e1]
            pre = main.tile([P, 2 * NCH], F32)
            mv = main.tile([P, 2], F32)
            mv3 = main.tile([P, 2], F32)
            t_a = main.tile([P, 1], F32)
            u_a = main.tile([P, 3, 2], F32)
            u_s = main.tile([P, 2], F32)  # [Sx, Sx^2] accumulators (chunk1)
            u_w = main.tile([P, 1], F32)
            SCL = 1.0 / (NB * HW * N_CORES)
            nc.vector.bn_aggr(mv[:], st6a[:])
            nc.vector.tensor_mul(t_a[:], mv[:, 0:1], mv[:, 0:1])
            nc.vector.tensor_scalar(
                pre[:, 1:2], mv[:, 1:2], t_a[:], 1.0 / N_CORES,
                op0=ALU.add, op1=ALU.mult,
            )
            nc.vector.tensor_scalar_mul(pre[:, 0:1], mv[:, 0:1], 1.0 / N_CORES)
            nc.vector.bn_aggr(mv3[:], st6b[:])
            sqv = sq_s.rearrange("p (n k) -> p n k", k=2)
            nc.vector.tensor_add(u_a[:], sqv[:, 0:3, :], sqv[:, 3:6, :])
            nc.vector.tensor_add(u_s[:], u_a[:, 0, :], u_a[:, 1, :])
            nc.vector.tensor_add(u_s[:], u_s[:], u_a[:, 2, :])
            # Sx_c1 += mean_b*NTAIL; Sq_c1 += (var_b+mean_b^2)*NTAIL
            nc.vector.scalar_tensor_tensor(
                u_s[:, 0:1], mv3[:, 0:1], NTAIL, u_s[:, 0:1],
                op0=ALU.mult, op1=ALU.add,
            )
            nc.vector.scalar_tensor_tensor(
                u_w[:], mv3[:, 0:1], mv3[:, 0:1], mv3[:, 1:2],
                op0=ALU.mult, op1=ALU.add,
            )
            nc.vector.scalar_tensor_tensor(
                u_s[:, 1:2], u_w[:], NTAIL, u_s[:, 1:2],
                op0=ALU.mult, op1=ALU.add,
            )
            nc.vector.tensor_scalar_mul(pre[:, 2:3], u_s[:, 0:1], SCL)
            nc.vector.tensor_scalar_mul(pre[:, 3:4], u_s[:, 1:2], SCL)
            # pre holds per-core (mean, E[x^2]) per chunk; the exchange sums
            # across cores, so scale by 1/8 AFTER the sum (done via SCL8 in
            # the threshold math below on the summed values /8).

            # ---- cross-core sum of `pre` via the CC AllReduce. The
            # collective firmware has a ~55-60us cold start from kernel
            # launch before the first mesh can begin, independent of the
            # trigger time (measured; a warm-up dummy collective does not
            # help, the real mesh just queues behind it; remote_dma
            # broadcast hangs in this environment), so this is the floor.
            cc_in = dram.tile([P, 2 * NCH], F32)
            nc.gpsimd.dma_start(cc_in[:], pre[:])
            nc.gpsimd.collective_compute(
                "AllReduce",
                ALU.add,
                replica_groups=[list(range(N_CORES))],
                ins=[cc_in[:].opt()],
                outs=[cc_sh[:, :].opt()],
            )
            gs = main.tile([P, 2 * NCH], F32)
            # result load on the sync queue (idle here; HWDGE beats the
            # gpsimd SWDGE issue latency)
            nc.sync.dma_start(gs[:], cc_sh[:, :])

            # thresholds: sign(gamma*(x-m)*rsqrt + beta) =
            # sign(gamma) * sign(x + (r*sigma - m)), r = beta/gamma
            # (host folds sign(gamma) into the weights, ships r in parc
            # slot 0). ab = r*sigma - m (ScalarE bias form); thr = -ab
            # (DVE compare form). gs holds the global (mean, E[x^2]).
            ab = main.tile([P, NCH], F32)
            thr = main.tile([P, NCH], F32)
            u1 = main.tile([P, NCH], F32)
            u2 = main.tile([P, NCH], F32)
            gsv = gs.rearrange("p (c s) -> p c s", s=2)
            gmean = gsv[:, :, 0]
            parv = parc.rearrange("p (c s) -> p c s", s=3)
            nc.vector.tensor_mul(u1[:], gmean, gmean)
            nc.vector.tensor_sub(u2[:], gsv[:, :, 1], u1[:])  # global var
            nc.scalar.activation(u1[:], u2[:], AF.Sqrt, bias=epsT[:, 0:1])
            nc.vector.tensor_mul(u2[:], parv[:, :, 0], u1[:])  # r*sigma
            nc.vector.tensor_sub(ab[:], u2[:], gmean)
            nc.vector.tensor_sub(thr[:], gmean, u2[:])

            # ---- normalize + sign -> padded planes, split by chunk:
            # ScalarE: chunk 0 via Sign activation (+-1)
            # DVE: chunk 1 via (x>thr)-0.5 (+-0.5; weights shipped x2).
            # Fine first slices so the first conv matmuls unblock ASAP.
            for n in range(NB):
                slices = ((0, 10), (10, 34), (34, H)) if n == 0 else ((0, 34), (34, H))
                for r0, r1 in slices:
                    nc.scalar.activation(
                        xbv[:, 0, n, 1 + r0:1 + r1, 1:1 + W],
                        xt[0][:, n * HW + r0 * W:n * HW + r1 * W]
                        .rearrange("p (h w) -> p h w", w=W),
                        AF.Sign,
                        bias=ab[:, 0:1],
                    )
                    nc.vector.tensor_scalar(
                        xbv[:, 1, n, 1 + r0:1 + r1, 1:1 + W],
                        xt[1][:, n * HW + r0 * W:n * HW + r1 * W]
                        .rearrange("p (h w) -> p h w", w=W),
                        thr[:, 1:2], 0.5,
                        op0=ALU.is_gt, op1=ALU.subtract,
                    )

            # ---- 3x3 binary conv, image-major block order (each image's
            # planes feed 14 jobs before the next image's are needed).
            # 7-job blocks leave one spare PSUM bank for block overlap.
            blocks = []
            for n in range(NB):
                for o in range(NCH):
                    jb = [(n, g) for g in range(NG)]
                    if n == 0 and o == 0:
                        # split the first block finely: [g0] runs off sign
                        # slice (0,10) alone, [g1-g3] off (10,34), the rest
                        # off (34,56)
                        blocks.append((o, jb[:1]))
                        blocks.append((o, jb[1:4]))
                        blocks.append((o, jb[4:]))
                    elif n == NB - 1 and o == NCH - 1:
                        # split the final block so its first drains+stores
                        # overlap the last jobs' matmuls, with a 1-job tail
                        blocks.append((o, jb[:4]))
                        blocks.append((o, jb[4:6]))
                        blocks.append((o, jb[6:]))
                    else:
                        blocks.append((o, jb))
            last_blk_idx = len(blocks) - 1
            for bi, (o, blk) in enumerate(blocks):
                pts = [psum.tile([P, NT], F32, name="ps", tag="ps") for _ in blk]
                for t in range(9):
                    ky, kx = divmod(t, 3)
                    w_ap = wb[:, :, (t * NCH + o) * P:(t * NCH + o + 1) * P]
                    for k, (n, g) in enumerate(blk):
                        rhs = xbv[:, :, n, g * RG + ky: g * RG + ky + RG, kx:kx + W]
                        mm = nc.tensor.matmul(
                            pts[k][:], w_ap, rhs,
                            start=(t == 0), stop=(t == 8),
                            perf_mode=mybir.MatmulPerfMode.DoubleRow,
                        )
                        if k > 0:
                            mm.ins.ldweights = False
                for k, (n, g) in enumerate(blk):
                    ob = outp.tile([P, NT], F16, name="ob", tag="ob")
                    # drains on ScalarE (own PSUM port) except the final
                    # block, where splitting across both engines halves
                    # the serialized drain tail
                    if bi == last_blk_idx:
                        nc.scalar.activation(ob[:, :NT // 2],
                                             pts[k][:, :NT // 2], AF.Relu,
                                             bias=parc[:, 3 * o + 2:3 * o + 3])
                        nc.vector.tensor_scalar(
                            ob[:, NT // 2:], pts[k][:, NT // 2:],
                            parc[:, 3 * o + 2:3 * o + 3], 0.0,
                            op0=ALU.add, op1=ALU.max,
                        )
                    else:
                        nc.scalar.activation(ob[:, :], pts[k][:, :], AF.Relu,
                                             bias=parc[:, 3 * o + 2:3 * o + 3])
                    # flat contiguous dest so the DMA coalesces full-line
                    # writes; single queue (dual-queue raced the drains)
                    nc.sync.dma_start(
                        ys[n, o * P:(o + 1) * P]
                        .rearrange("p h w -> p (h w)")[:, g * NT:(g + 1) * NT],
                        ob[:],
                    )
    nc.compile()
    return nc


def _get_nc():
    if "nc" not in _CACHE:
        _CACHE["nc"] = _build_nc()
    return _CACHE["nc"]


def _prep_inputs(x, gamma, beta, weight, bias):
    # fold sign(gamma) (per input channel) into the binarized weights so the
    # device computes just sign(x - t); r = beta/gamma feeds the threshold
    gamma = np.asarray(gamma, dtype=np.float32)
    beta = np.asarray(beta, dtype=np.float32)
    sg = np.sign(gamma)
    r = np.divide(beta, gamma, out=np.zeros_like(beta), where=gamma != 0)
    wsign = np.sign(weight.astype(np.float32)) * sg[None, :, None, None]
    # x2 on weights whose input channels are DVE-signed (+-0.5 encoding):
    # all of chunk 1
    fac = np.ones((NCH, P), dtype=np.float32)
    fac[1, :] = 2.0
    wT = (
        wsign.reshape(NCH, P, NCH, P, 3, 3)      # o, m, c, p, ky, kx
        * fac[None, None, :, :, None, None]
    )
    wT = (
        wT.transpose(3, 2, 4, 5, 0, 1)           # p, c, ky, kx, o, m
        .reshape(P, NCH, 9 * NCH * P)
        .astype(mybir.dt.np(FP8))
    )
    par = np.stack(
        [r, beta, bias.astype(np.float32)],
        axis=-1,
    ).reshape(NCH, P, 3)
    x = np.ascontiguousarray(np.asarray(x, dtype=np.float32).astype(np.float16))
    return [
        {"xs": x[j * NB:(j + 1) * NB], "wt": wT, "par": par}
        for j in range(N_CORES)
    ]


def _run(x, gamma, beta, weight, bias, trace=False, trace_cores=None):
    nc = _get_nc()
    in_maps = _prep_inputs(x, gamma, beta, weight, bias)
    res = bass_utils.run_bass_kernel_spmd(
        nc, in_maps, core_ids=list(range(N_CORES)), trace=trace,
        trace_cores=trace_cores,
    )
    out = np.concatenate([res.results[j]["ys"] for j in range(N_CORES)], axis=0)
    return out.astype(np.float32), res


def kernel(x, gamma, beta, weight, bias):
    out, _ = _run(x, gamma, beta, weight, bias, trace=False)
    return out
